# revision 4
# baseline (speedup 1.0000x reference)
"""Trainium2 Bass kernel for GQA causal attention (dense_transformer).

Module: x:[2,2048,1024] -> fused QKV proj (16 Q heads, 4 KV heads, D=64,
only first 1536 rows of w_qkv used) -> causal GQA attention -> out proj.

Sharding (8 NeuronCores): core c = (batch b=c//4, TP rank r=c%4).
Each core owns batch b, query heads 4r..4r+3 and GQA KV head r.
 - QKV projection column-parallel (per-rank weight slices, host-sliced).
 - Attention fully local (GQA group == rank's 4 query heads + 1 KV head).
 - Attention output (transposed layout [256, S]) AllGather'd across the
   4-rank TP group -> [1024, S]; output projection column-parallel
   (each rank computes 256 output features); host concatenates.

On-device layout notes:
 - Everything runs in "transposed" [feature, seq] layout so the TensorE
   contractions need no on-device transposes.
 - Softmax without running max (scores ~ N(0,1) after scale, exp is safe).
 - Rowsum via ones-matrix matmul (broadcast across partitions for free).
 - float32r matmuls: 4x faster than float32 on the PE at ~1e-4 rel err.
"""

import os
import sys

import numpy as np

if "/opt/trn_rl_repo" not in sys.path:
    sys.path.insert(0, "/opt/trn_rl_repo")

B = 2
S = 2048
LATENT = 1024
H = 16
HK = 4
D = 64
NCORES = 8
TP = 4           # tensor-parallel ranks per batch
QH = H // TP     # query heads per core
DQ = QH * D      # 256 attention features per core
SCALE = 1.0 / 8.0
QBLK = 512
NQB = S // QBLK  # 4
KT = 128
NKT = S // KT    # 16
LCH = LATENT // 128  # 8 contraction chunks

_CACHE = {}


def _build():
    import concourse.bacc as bacc
    from concourse import mybir
    from concourse.tile import TileContext

    f32 = mybir.dt.float32
    f32r = mybir.dt.float32r
    Exp = mybir.ActivationFunctionType.Exp

    nc = bacc.Bacc("TRN2", target_bir_lowering=False, num_devices=NCORES)

    x_t = nc.declare_dram_parameter("x_t", [LATENT, S], f32r, isOutput=False)
    wq_t = nc.declare_dram_parameter("wq_t", [LATENT, DQ], f32r, isOutput=False)
    wk_d = nc.declare_dram_parameter("wk_d", [LATENT, 128], f32r, isOutput=False)
    wv_t = nc.declare_dram_parameter("wv_t", [LATENT, D], f32r, isOutput=False)
    wo_t = nc.declare_dram_parameter("wo_t", [LATENT, DQ], f32r, isOutput=False)
    mask = nc.declare_dram_parameter("mask", [128, 4 * 1024], f32, isOutput=False)
    out = nc.declare_dram_parameter("out", [DQ, S], f32, isOutput=True)

    RG = [[0, 1, 2, 3], [4, 5, 6, 7]]

    with TileContext(nc) as tc:
        with (
            tc.tile_pool(name="const", bufs=1) as cst,
            tc.tile_pool(name="sb", bufs=1) as sb,
            tc.tile_pool(name="ps", bufs=1, space="PSUM") as ps,
            tc.tile_pool(name="dram", bufs=1, space="DRAM") as dram,
        ):
            # ---- constants / weights ----
            ones_f = cst.tile([128, 64], f32)
            nc.vector.memset(ones_f[:], 1.0)
            ones_r = cst.tile([128, 64], f32r)
            nc.scalar.copy(ones_r[:], ones_f[:])
            # preload the exp table set early (overlaps weight DMAs)
            dummy = cst.tile([128, 8], f32)
            nc.scalar.activation(dummy[:], ones_f[:, :8], Exp)

            mask_sb = cst.tile([128, 4 * 1024], f32)
            nc.sync.dma_start(mask_sb[:], mask[:])
            wq_sb = cst.tile([128, LCH, DQ], f32r)
            nc.sync.dma_start(wq_sb[:], wq_t[:].rearrange("(l p) m -> p l m", p=128))
            wk_sb = cst.tile([128, LCH, 128], f32r)
            nc.sync.dma_start(wk_sb[:], wk_d[:].rearrange("(l p) m -> p l m", p=128))
            wv_sb = cst.tile([128, LCH, D], f32r)
            nc.sync.dma_start(wv_sb[:], wv_t[:].rearrange("(l p) m -> p l m", p=128))
            wo_sb = cst.tile([128, LCH, DQ], f32r)
            nc.sync.dma_start(wo_sb[:], wo_t[:].rearrange("(l p) m -> p l m", p=128))

            # ---- persistent activations ----
            qT0 = sb.tile([128, S], f32r)   # heads 0,1 (rows 0:64 / 64:128)
            qT1 = sb.tile([128, S], f32r)   # heads 2,3
            qT_sb = [qT0, qT1]
            kT_sb = sb.tile([128, S], f32r)  # duplicated kT (rows 64:128 copy)
            # v tile t at [:, t, 0:64] (seq-major); [:, t, 64:128] = ones so the
            # fused OT matmul also produces the softmax rowsum in rows 64:128
            v_sb = sb.tile([128, NKT, 128], f32r)
            for t in range(NKT):
                nc.vector.tensor_copy(v_sb[:, t, 64:128], ones_f[:])

            # DRAM bounce buffers for the per-qblock AllGather
            agin = [dram.tile([DQ, QBLK], f32r, name=f"agin{j}") for j in range(NQB)]
            agout = [dram.tile([TP * DQ, QBLK], f32r, name=f"agout{j}")
                     for j in range(NQB)]

            # ---- phase 1: QKV projection (per q block) ----
            for j in range(NQB):
                qs = slice(QBLK * j, QBLK * (j + 1))
                xc = sb.tile([128, LCH, QBLK], f32r, tag="xc", bufs=2)
                nc.sync.dma_start(
                    xc[:], x_t[:].rearrange("(l p) s -> p l s", p=128)[:, :, qs]
                )
                for c in range(2):
                    qps = ps.tile([128, QBLK], f32, tag="mm512", bufs=2)
                    for l in range(LCH):
                        nc.tensor.matmul(
                            qps[:],
                            wq_sb[:, l, 128 * c:128 * (c + 1)],
                            xc[:, l, :],
                            start=(l == 0),
                            stop=(l == LCH - 1),
                        )
                    nc.vector.tensor_copy(qT_sb[c][:, qs], qps[:])
                kps = ps.tile([128, QBLK], f32, tag="mm512", bufs=2)
                for l in range(LCH):
                    nc.tensor.matmul(
                        kps[:],
                        wk_sb[:, l, :],
                        xc[:, l, :],
                        start=(l == 0),
                        stop=(l == LCH - 1),
                    )
                nc.vector.tensor_copy(kT_sb[:, qs], kps[:])
                for si in range(QBLK // 128):
                    st_glob = (QBLK // 128) * j + si
                    vps = ps.tile([128, D], f32, tag="mm512", bufs=2)
                    for l in range(LCH):
                        nc.tensor.matmul(
                            vps[:],
                            xc[:, l, 128 * si:128 * (si + 1)],
                            wv_sb[:, l, :],
                            start=(l == 0),
                            stop=(l == LCH - 1),
                        )
                    nc.vector.tensor_copy(v_sb[:, st_glob, 0:D], vps[:])

            # ---- phase 2+3: attention, AllGather, output projection ----
            for j in range(NQB):
                qs = slice(QBLK * j, QBLK * (j + 1))
                nkt_j = (QBLK // KT) * (j + 1)
                for p in range(2):
                    # fused OT+rowsum accumulators, one bank per head:
                    # rows 0:64 = V.T @ P.T (attention out), 64:128 = rowsum
                    oa = ps.tile([128, QBLK], f32, tag="otrs", bufs=2)
                    ob = ps.tile([128, QBLK], f32, tag="otrs", bufs=2)
                    for t in range(nkt_j):
                        ks = slice(KT * t, KT * (t + 1))
                        st = ps.tile([128, 2 * QBLK], f32, tag="st", bufs=2)
                        nc.tensor.matmul(
                            st[:, 0:QBLK], kT_sb[0:64, ks], qT_sb[p][0:64, qs],
                            start=True, stop=True, tile_position=(0, 0),
                        )
                        nc.tensor.matmul(
                            st[:, QBLK:2 * QBLK], kT_sb[64:128, ks],
                            qT_sb[p][64:128, qs],
                            start=True, stop=True, tile_position=(64, 0),
                        )
                        tl = t - (QBLK // KT) * j  # >=0 on the diagonal strip
                        if tl >= 0:
                            ptr = sb.tile([128, 2 * QBLK], f32, tag="ptraw", bufs=2)
                            nc.scalar.activation(ptr[:], st[:], Exp, scale=SCALE)
                            pt = sb.tile([128, 2 * QBLK], f32r, tag="pt", bufs=3)
                            nc.vector.tensor_mul(
                                pt[:], ptr[:],
                                mask_sb[:, 1024 * tl:1024 * (tl + 1)],
                            )
                        else:
                            pt = sb.tile([128, 2 * QBLK], f32r, tag="pt", bufs=3)
                            nc.scalar.activation(pt[:], st[:], Exp, scale=SCALE)
                        first, last = (t == 0), (t == nkt_j - 1)
                        nc.tensor.matmul(
                            oa[:], v_sb[:, t, :], pt[:, 0:QBLK],
                            start=first, stop=last,
                        )
                        nc.tensor.matmul(
                            ob[:], v_sb[:, t, :], pt[:, QBLK:2 * QBLK],
                            start=first, stop=last,
                        )
                    for hb, oh in ((0, oa), (1, ob)):
                        h = 2 * p + hb
                        rcp = sb.tile([128, QBLK], f32, tag="rcp", bufs=2)
                        nc.vector.reciprocal(rcp[64:128, :], oh[64:128, :])
                        # cross-partition move 64:128 -> 0:64 (DMA only path)
                        nc.sync.dma_start(rcp[0:64, :], rcp[64:128, :])
                        apc = sb.tile([64, QBLK], f32r, tag="apc", bufs=2)
                        nc.vector.tensor_mul(apc[:], oh[0:64, :], rcp[0:64, :])
                        nc.sync.dma_start(agin[j][64 * h:64 * (h + 1), :], apc[:])

                nc.gpsimd.collective_compute(
                    "AllGather",
                    mybir.AluOpType.bypass,
                    replica_groups=RG,
                    ins=[agin[j][:].opt()],
                    outs=[agout[j][:].opt()],
                )
                agsb = sb.tile([128, LCH, QBLK], f32r, tag="agsb", bufs=2)
                nc.sync.dma_start(
                    agsb[:], agout[j][:].rearrange("(l p) s -> p l s", p=128)
                )
                for n in range(2):
                    pp = ps.tile([128, QBLK], f32, tag="mm512", bufs=2)
                    for i in range(LCH):
                        nc.tensor.matmul(
                            pp[:],
                            wo_sb[:, i, 128 * n:128 * (n + 1)],
                            agsb[:, i, :],
                            start=(i == 0),
                            stop=(i == LCH - 1),
                        )
                    opc = sb.tile([128, QBLK], f32, tag="opc", bufs=2)
                    nc.vector.tensor_copy(opc[:], pp[:])
                    nc.sync.dma_start(out[128 * n:128 * (n + 1), qs], opc[:])

    nc.finalize()
    return nc


def _shard_inputs(x, w_qkv, w_out):
    """Build the per-core input maps (host-side sharding only)."""
    x = np.asarray(x, dtype=np.float32)
    w_qkv = np.asarray(w_qkv, dtype=np.float32)
    w_out = np.asarray(w_out, dtype=np.float32)

    # causal masks for the 4 diagonal k-tile offsets, replicated for the
    # two heads packed side by side in each 1024-wide strip
    kk = np.arange(128)[:, None]
    qq = np.arange(QBLK)[None, :]
    strips = []
    for t in range(4):
        m = (kk <= qq - 128 * t).astype(np.float32)  # [128, 512]
        strips.append(np.concatenate([m, m], axis=1))  # [128, 1024]
    mask = np.ascontiguousarray(np.concatenate(strips, axis=1))  # [128, 4096]

    in_maps = []
    for c in range(NCORES):
        b, r = divmod(c, TP)
        wq = w_qkv[DQ * r:DQ * (r + 1), :]                    # [256, 1024]
        wk = w_qkv[H * D + D * r:H * D + D * (r + 1), :]      # [64, 1024]
        wv = w_qkv[(H + HK) * D + D * r:(H + HK) * D + D * (r + 1), :]
        wo = w_out[DQ * r:DQ * (r + 1), :]                    # [256, 1024]
        in_maps.append({
            "x_t": np.ascontiguousarray(x[b].T),
            "wq_t": np.ascontiguousarray(wq.T),
            "wk_d": np.ascontiguousarray(
                np.concatenate([wk.T, wk.T], axis=1)),
            "wv_t": np.ascontiguousarray(wv.T),
            "wo_t": np.ascontiguousarray(wo.T),
            "mask": mask,
        })
    return in_maps


def _get_nc():
    if "nc" not in _CACHE:
        _CACHE["nc"] = _build()
    return _CACHE["nc"]


def _install_ntff_shim():
    """Make BASS_TRACE work under axon (antenv.axon_hooks is absent here)."""
    import types
    if "antenv.axon_hooks" in sys.modules:
        return True
    try:
        import antenv
        from trn_agent_boot.trn_boot import _ntff_profile_via_ctypes
        hook = _ntff_profile_via_ctypes("/opt/axon/libaxon_pjrt.so")
        if hook is None:
            return False
        mod = types.ModuleType("antenv.axon_hooks")
        state = {"hook": hook}
        mod.set_axon_ntff_profile_hook = lambda h: state.__setitem__("hook", h)
        mod.get_axon_ntff_profile_hook = lambda: state["hook"]
        sys.modules["antenv.axon_hooks"] = mod
        antenv.axon_hooks = mod
        return True
    except Exception:
        return False


LAST_RESULT = None


def kernel(x, w_qkv, w_out):
    global LAST_RESULT
    from concourse.bass_utils import run_bass_kernel_spmd

    nc = _get_nc()
    in_maps = _shard_inputs(x, w_qkv, w_out)

    trace = bool(os.environ.get("BASS_TRACE"))
    if trace:
        trace = _install_ntff_shim()
    kwargs = {}
    if trace and os.environ.get("BASS_TRACE_CORES") == "all":
        kwargs["trace_cores"] = list(range(NCORES))
    res = run_bass_kernel_spmd(
        nc, in_maps, core_ids=list(range(NCORES)), trace=trace, **kwargs
    )
    LAST_RESULT = res

    full = np.empty((B, S, LATENT), dtype=np.float32)
    for c in range(NCORES):
        b, r = divmod(c, TP)
        full[b, :, DQ * r:DQ * (r + 1)] = res.results[c]["out"].T
    return full


# revision 11
# speedup vs baseline: 1.3030x; 1.3030x over previous
"""Trainium2 Bass kernel for GQA causal attention (dense_transformer).

Module: x:[2,2048,1024] -> fused QKV proj (16 Q heads, 4 KV heads, D=64,
only first 1536 rows of w_qkv used) -> causal GQA attention -> out proj.

Sharding (8 NeuronCores): core c = (batch b=c//4, TP rank r=c%4).
Each core owns batch b, query heads 4r..4r+3 and GQA KV head r.
 - QKV projection column-parallel (per-rank weight slices, host-sliced).
 - Attention fully local (GQA group == rank's 4 query heads + 1 KV head).
 - Attention output (transposed layout [256, S]) AllGather'd across the
   4-rank TP group -> [1024, S]; output projection column-parallel
   (each rank computes 256 output features); host concatenates.

On-device layout notes:
 - Everything runs in "transposed" [feature, seq] layout so the TensorE
   contractions need no on-device transposes.
 - Softmax without running max (scores ~ N(0,1) after scale, exp is safe).
 - Rowsum via ones-matrix matmul (broadcast across partitions for free).
 - float32r matmuls: 4x faster than float32 on the PE at ~1e-4 rel err.
"""

import os
import sys

import numpy as np

if "/opt/trn_rl_repo" not in sys.path:
    sys.path.insert(0, "/opt/trn_rl_repo")

B = 2
S = 2048
LATENT = 1024
H = 16
HK = 4
D = 64
NCORES = 8
TP = 4           # tensor-parallel ranks per batch
QH = H // TP     # query heads per core
DQ = QH * D      # 256 attention features per core
SCALE = 1.0 / 8.0
QBLK = 512
NQB = S // QBLK  # 4
KT = 128
NKT = S // KT    # 16
LCH = LATENT // 128  # 8 contraction chunks

_CACHE = {}
DEBUG = False


def _build():
    import concourse.bacc as bacc
    from concourse import mybir
    from concourse.tile import TileContext

    f32 = mybir.dt.float32
    f32r = mybir.dt.float32r
    Exp = mybir.ActivationFunctionType.Exp

    nc = bacc.Bacc("TRN2", target_bir_lowering=False, num_devices=NCORES)

    x_t = nc.declare_dram_parameter("x_t", [LATENT, S], f32r, isOutput=False)
    wq_t = nc.declare_dram_parameter("wq_t", [LATENT, DQ], f32r, isOutput=False)
    wk_d = nc.declare_dram_parameter("wk_d", [LATENT, 128], f32r, isOutput=False)
    wv_t = nc.declare_dram_parameter("wv_t", [LATENT, D], f32r, isOutput=False)
    wo_t = nc.declare_dram_parameter("wo_t", [LATENT, DQ], f32r, isOutput=False)
    mask = nc.declare_dram_parameter("mask", [128, 4 * 1024], f32r, isOutput=False)
    eye = nc.declare_dram_parameter("eye", [64, 64], f32, isOutput=False)
    out = nc.declare_dram_parameter("out", [DQ, S], f32, isOutput=True)
    dbg = {}
    if DEBUG:
        for nm, shp in (("dbg_pt", [128, 1024]), ("dbg_oa", [128, 512]),
                        ("dbg_rsm", [64, 512]), ("dbg_rcp", [64, 512]),
                        ("dbg_apc", [64, 512]), ("dbg_v", [128, 128]),
                        ("dbg_st", [128, 1024])):
            dbg[nm] = nc.declare_dram_parameter(nm, shp, f32, isOutput=True)

    RG = [[0, 1, 2, 3], [4, 5, 6, 7]]

    with TileContext(nc) as tc:
        with (
            tc.tile_pool(name="const", bufs=1) as cst,
            tc.tile_pool(name="sb", bufs=1) as sb,
            tc.tile_pool(name="ps", bufs=1, space="PSUM") as ps,
            tc.tile_pool(name="dram", bufs=1, space="DRAM") as dram,
        ):
            # ---- constants / weights ----
            ones_f = cst.tile([128, 64], f32)
            nc.vector.memset(ones_f[:], 1.0)
            ones_r = cst.tile([128, 64], f32r)
            nc.scalar.copy(ones_r[:], ones_f[:])
            # preload the exp table set early (overlaps weight DMAs)
            dummy = cst.tile([128, 8], f32)
            nc.scalar.activation(dummy[:], ones_f[:, :8], Exp)

            mask_sb = cst.tile([128, 4 * 1024], f32r)
            nc.sync.dma_start(mask_sb[:], mask[:])
            eye_sb = cst.tile([64, 64], f32)
            nc.sync.dma_start(eye_sb[:], eye[:])
            wq_sb = cst.tile([128, LCH, DQ], f32r)
            nc.sync.dma_start(wq_sb[:], wq_t[:].rearrange("(l p) m -> p l m", p=128))
            wk_sb = cst.tile([128, LCH, 128], f32r)
            nc.sync.dma_start(wk_sb[:], wk_d[:].rearrange("(l p) m -> p l m", p=128))
            wv_sb = cst.tile([128, LCH, D], f32r)
            nc.sync.dma_start(wv_sb[:], wv_t[:].rearrange("(l p) m -> p l m", p=128))
            wo_sb = cst.tile([128, LCH, DQ], f32r)
            nc.sync.dma_start(wo_sb[:], wo_t[:].rearrange("(l p) m -> p l m", p=128))

            # ---- persistent activations ----
            qT0 = sb.tile([128, S], f32r)   # heads 0,1 (rows 0:64 / 64:128)
            qT1 = sb.tile([128, S], f32r)   # heads 2,3
            qT_sb = [qT0, qT1]
            kT_sb = sb.tile([128, S], f32r)  # duplicated kT (rows 64:128 copy)
            # v tile t at [:, t, 0:64] (seq-major); [:, t, 64:128] = ones so the
            # fused OT matmul also produces the softmax rowsum in rows 64:128
            v_sb = sb.tile([128, NKT, 128], f32r)
            for t in range(NKT):
                nc.vector.tensor_copy(v_sb[:, t, 64:128], ones_f[:])

            # DRAM bounce buffers for the per-qblock AllGather
            agin = [dram.tile([DQ, QBLK], f32r, name=f"agin{j}") for j in range(NQB)]
            agout = [dram.tile([TP * DQ, QBLK], f32r, name=f"agout{j}")
                     for j in range(NQB)]

            # ---- phase 1: QKV projection (per q block) ----
            for j in range(NQB):
                qs = slice(QBLK * j, QBLK * (j + 1))
                xc = sb.tile([128, LCH, QBLK], f32r, tag="xc", bufs=2)
                xr = x_t[:].rearrange("(l p) s -> p l s", p=128)
                for l in range(LCH):
                    nc.sync.dma_start(xc[:, l, :], xr[:, l, qs])
                for c in range(2):
                    qps = ps.tile([128, QBLK], f32, tag="mm512", bufs=2)
                    for l in range(LCH):
                        nc.tensor.matmul(
                            qps[:],
                            wq_sb[:, l, 128 * c:128 * (c + 1)],
                            xc[:, l, :],
                            start=(l == 0),
                            stop=(l == LCH - 1),
                        )
                    nc.vector.tensor_copy(qT_sb[c][:, qs], qps[:])
                kps = ps.tile([128, QBLK], f32, tag="mm512", bufs=2)
                for l in range(LCH):
                    nc.tensor.matmul(
                        kps[:],
                        wk_sb[:, l, :],
                        xc[:, l, :],
                        start=(l == 0),
                        stop=(l == LCH - 1),
                    )
                nc.vector.tensor_copy(kT_sb[:, qs], kps[:])
                vtp = ps.tile([128, QBLK], f32, tag="mm512", bufs=2)
                for l in range(LCH):
                    nc.tensor.matmul(
                        vtp[0:D, :],
                        wv_sb[:, l, :],
                        xc[:, l, :],
                        start=(l == 0),
                        stop=(l == LCH - 1),
                    )
                vt_sb = sb.tile([64, QBLK], f32, tag="vt", bufs=2)
                nc.vector.tensor_copy(vt_sb[:], vtp[0:D, :])
                for si in range(QBLK // 128):
                    st_glob = (QBLK // 128) * j + si
                    vps = ps.tile([128, D], f32, tag="mm512", bufs=2)
                    nc.tensor.transpose(
                        vps[:], vt_sb[:, 128 * si:128 * (si + 1)], eye_sb[:]
                    )
                    nc.vector.tensor_copy(v_sb[:, st_glob, 0:D], vps[:])
                    if DEBUG and st_glob == 0:
                        vf = sb.tile([128, 128], f32, name="vf")
                        nc.vector.tensor_copy(vf[:], v_sb[:, 0, :])
                        nc.sync.dma_start(dbg["dbg_v"][:], vf[:])

            # ---- phase 2+3: attention, AllGather, output projection ----
            # Emission order pipelines proj(j-1) AFTER attention(j): the PE
            # instruction queue is FIFO, so proj matmuls must sit behind the
            # next attention block or every AllGather stalls the PE.
            def attention(j):
                qs = slice(QBLK * j, QBLK * (j + 1))
                nkt_j = (QBLK // KT) * (j + 1)
                for p in range(2):
                    # fused OT+rowsum accumulators, one bank per head:
                    # rows 0:64 = V.T @ P.T (attention out), 64:128 = rowsum
                    oa = ps.tile([128, QBLK], f32, tag="otrs", bufs=2, name="oa")
                    ob = ps.tile([128, QBLK], f32, tag="otrs", bufs=2, name="ob")
                    for t in range(nkt_j):
                        ks = slice(KT * t, KT * (t + 1))
                        st = ps.tile([128, 2 * QBLK], f32, tag="st", bufs=2,
                                     name="st")
                        nc.tensor.matmul(
                            st[:, 0:QBLK], kT_sb[0:64, ks], qT_sb[p][0:64, qs],
                            start=True, stop=True, tile_position=(0, 0),
                        )
                        nc.tensor.matmul(
                            st[:, QBLK:2 * QBLK], kT_sb[64:128, ks],
                            qT_sb[p][64:128, qs],
                            start=True, stop=True, tile_position=(64, 0),
                        )
                        if DEBUG and j == 0 and p == 0 and t == 0:
                            stf = sb.tile([128, 2 * QBLK], f32, name="stf")
                            nc.vector.tensor_copy(stf[:], st[:])
                            nc.sync.dma_start(dbg["dbg_st"][:], stf[:])
                        tl = t - (QBLK // KT) * j  # >=0 on the diagonal strip
                        if tl >= 0:
                            ptr = sb.tile([128, 2 * QBLK], f32r, tag="ptraw",
                                          bufs=2, name="ptr")
                            nc.scalar.activation(ptr[:], st[:], Exp, scale=SCALE)
                            pt = sb.tile([128, 2 * QBLK], f32r, tag="pt", bufs=4,
                                         name="pt")
                            nc.vector.tensor_mul(
                                pt[:], ptr[:],
                                mask_sb[:, 1024 * tl:1024 * (tl + 1)],
                            )
                        else:
                            pt = sb.tile([128, 2 * QBLK], f32r, tag="pt", bufs=4,
                                         name="pt")
                            nc.scalar.activation(pt[:], st[:], Exp, scale=SCALE)
                        if DEBUG and j == 0 and p == 0 and t == 0:
                            ptf = sb.tile([128, 2 * QBLK], f32, name="ptf")
                            nc.vector.tensor_copy(ptf[:], pt[:])
                            nc.sync.dma_start(dbg["dbg_pt"][:], ptf[:])
                        first, last = (t == 0), (t == nkt_j - 1)
                        nc.tensor.matmul(
                            oa[:], v_sb[:, t, :], pt[:, 0:QBLK],
                            start=first, stop=last,
                        )
                        nc.tensor.matmul(
                            ob[:], v_sb[:, t, :], pt[:, QBLK:2 * QBLK],
                            start=first, stop=last,
                        )
                    if DEBUG and j == 0 and p == 0:
                        oaf = sb.tile([128, QBLK], f32, name="oaf")
                        nc.vector.tensor_copy(oaf[:], oa[:])
                        nc.sync.dma_start(dbg["dbg_oa"][:], oaf[:])
                    for hb, oh in ((0, oa), (1, ob)):
                        h = 2 * p + hb
                        rsm = sb.tile([128, QBLK], f32, tag="rsm", bufs=2,
                                      name="rsm")
                        # psum -> sbuf first: custom DVE ops can't read PSUM
                        nc.vector.tensor_copy(rsm[64:128, :], oh[64:128, :])
                        # cross-partition move 64:128 -> 0:64 (DMA only path);
                        # approx-recip then runs at base partition 0 (it
                        # mis-executes on base-64 slices)
                        nc.sync.dma_start(rsm[0:64, :], rsm[64:128, :])
                        rcp = sb.tile([128, QBLK], f32, tag="rcp", bufs=2,
                                      name="rcp")
                        nc.vector.reciprocal_approx_fast(
                            out=rcp[0:64, :], in_=rsm[0:64, :])
                        if DEBUG and j == 0 and h == 0:
                            nc.sync.dma_start(dbg["dbg_rsm"][:], rsm[64:128, :])
                        apc = sb.tile([64, QBLK], f32r, tag="apc", bufs=2,
                                      name="apc")
                        nc.vector.tensor_mul(apc[:], oh[0:64, :], rcp[0:64, :])
                        if DEBUG and j == 0 and h == 0:
                            nc.sync.dma_start(dbg["dbg_rcp"][:], rcp[0:64, :])
                            apf = sb.tile([64, QBLK], f32, name="apf")
                            nc.vector.tensor_copy(apf[:], apc[:])
                            nc.sync.dma_start(dbg["dbg_apc"][:], apf[:])
                        nc.sync.dma_start(agin[j][64 * h:64 * (h + 1), :], apc[:])

                nc.gpsimd.collective_compute(
                    "AllGather",
                    mybir.AluOpType.bypass,
                    replica_groups=RG,
                    ins=[agin[j][:].opt()],
                    outs=[agout[j][:].opt()],
                )

            def out_proj(j):
                qs = slice(QBLK * j, QBLK * (j + 1))
                agsb = sb.tile([128, LCH, QBLK], f32r, tag="agsb", bufs=2,
                               name="agsb")
                agr = agout[j][:].rearrange("(l p) s -> p l s", p=128)
                for l in range(LCH):
                    nc.sync.dma_start(agsb[:, l, :], agr[:, l, :])
                for n in range(2):
                    pp = ps.tile([128, QBLK], f32, tag="mm512", bufs=2, name="pp")
                    for i in range(LCH):
                        nc.tensor.matmul(
                            pp[:],
                            wo_sb[:, i, 128 * n:128 * (n + 1)],
                            agsb[:, i, :],
                            start=(i == 0),
                            stop=(i == LCH - 1),
                        )
                    opc = sb.tile([128, QBLK], f32, tag="opc", bufs=2, name="opc")
                    nc.vector.tensor_copy(opc[:], pp[:])
                    nc.sync.dma_start(out[128 * n:128 * (n + 1), qs], opc[:])

            attention(0)
            attention(1)
            out_proj(0)
            attention(2)
            out_proj(1)
            attention(3)
            out_proj(2)
            out_proj(3)

    nc.finalize()
    return nc


def _shard_inputs(x, w_qkv, w_out):
    """Build the per-core input maps (host-side sharding only)."""
    x = np.asarray(x, dtype=np.float32)
    w_qkv = np.asarray(w_qkv, dtype=np.float32)
    w_out = np.asarray(w_out, dtype=np.float32)

    # causal masks for the 4 diagonal k-tile offsets, replicated for the
    # two heads packed side by side in each 1024-wide strip
    kk = np.arange(128)[:, None]
    qq = np.arange(QBLK)[None, :]
    strips = []
    for t in range(4):
        m = (kk <= qq - 128 * t).astype(np.float32)  # [128, 512]
        strips.append(np.concatenate([m, m], axis=1))  # [128, 1024]
    mask = np.ascontiguousarray(np.concatenate(strips, axis=1))  # [128, 4096]

    in_maps = []
    for c in range(NCORES):
        b, r = divmod(c, TP)
        wq = w_qkv[DQ * r:DQ * (r + 1), :]                    # [256, 1024]
        wk = w_qkv[H * D + D * r:H * D + D * (r + 1), :]      # [64, 1024]
        wv = w_qkv[(H + HK) * D + D * r:(H + HK) * D + D * (r + 1), :]
        wo = w_out[DQ * r:DQ * (r + 1), :]                    # [256, 1024]
        in_maps.append({
            "eye": np.eye(64, dtype=np.float32),
            "x_t": np.ascontiguousarray(x[b].T),
            "wq_t": np.ascontiguousarray(wq.T),
            "wk_d": np.ascontiguousarray(
                np.concatenate([wk.T, wk.T], axis=1)),
            "wv_t": np.ascontiguousarray(wv.T),
            "wo_t": np.ascontiguousarray(wo.T),
            "mask": mask,
        })
    return in_maps


def _get_nc():
    if "nc" not in _CACHE:
        _CACHE["nc"] = _build()
    return _CACHE["nc"]


def _install_ntff_shim():
    """Make BASS_TRACE work under axon (antenv.axon_hooks is absent here)."""
    import types
    if "antenv.axon_hooks" in sys.modules:
        return True
    try:
        import antenv
        from trn_agent_boot.trn_boot import _ntff_profile_via_ctypes
        hook = _ntff_profile_via_ctypes("/opt/axon/libaxon_pjrt.so")
        if hook is None:
            return False
        mod = types.ModuleType("antenv.axon_hooks")
        state = {"hook": hook}
        mod.set_axon_ntff_profile_hook = lambda h: state.__setitem__("hook", h)
        mod.get_axon_ntff_profile_hook = lambda: state["hook"]
        sys.modules["antenv.axon_hooks"] = mod
        antenv.axon_hooks = mod
        return True
    except Exception:
        return False


LAST_RESULT = None


def kernel(x, w_qkv, w_out):
    global LAST_RESULT
    from concourse.bass_utils import run_bass_kernel_spmd

    nc = _get_nc()
    in_maps = _shard_inputs(x, w_qkv, w_out)

    trace = bool(os.environ.get("BASS_TRACE"))
    if trace:
        trace = _install_ntff_shim()
    kwargs = {}
    if trace and os.environ.get("BASS_TRACE_CORES") == "all":
        kwargs["trace_cores"] = list(range(NCORES))
    res = run_bass_kernel_spmd(
        nc, in_maps, core_ids=list(range(NCORES)), trace=trace, **kwargs
    )
    LAST_RESULT = res

    full = np.empty((B, S, LATENT), dtype=np.float32)
    for c in range(NCORES):
        b, r = divmod(c, TP)
        full[b, :, DQ * r:DQ * (r + 1)] = res.results[c]["out"].T
    return full


# revision 12
# speedup vs baseline: 1.5586x; 1.1962x over previous
"""Trainium2 Bass kernel for GQA causal attention (dense_transformer).

Module: x:[2,2048,1024] -> fused QKV proj (16 Q heads, 4 KV heads, D=64,
only first 1536 rows of w_qkv used) -> causal GQA attention -> out proj.

Sharding (8 NeuronCores): core c = (batch b=c//4, TP rank r=c%4).
Each core owns batch b, query heads 4r..4r+3 and GQA KV head r.
 - QKV projection column-parallel (per-rank weight slices, host-sliced).
 - Attention fully local (GQA group == rank's 4 query heads + 1 KV head).
 - Attention output (transposed layout [256, S]) AllGather'd across the
   4-rank TP group -> [1024, S]; output projection column-parallel
   (each rank computes 256 output features); host concatenates.

On-device layout notes:
 - Everything runs in "transposed" [feature, seq] layout so the TensorE
   contractions need no on-device transposes.
 - Softmax without running max (scores ~ N(0,1) after scale, exp is safe).
 - Rowsum via ones-matrix matmul (broadcast across partitions for free).
 - float32r matmuls: 4x faster than float32 on the PE at ~1e-4 rel err.
"""

import os
import sys

import numpy as np
import ml_dtypes

if "/opt/trn_rl_repo" not in sys.path:
    sys.path.insert(0, "/opt/trn_rl_repo")

B = 2
S = 2048
LATENT = 1024
H = 16
HK = 4
D = 64
NCORES = 8
TP = 4           # tensor-parallel ranks per batch
QH = H // TP     # query heads per core
DQ = QH * D      # 256 attention features per core
SCALE = 1.0 / 8.0
QBLK = 512
NQB = S // QBLK  # 4
KT = 128
NKT = S // KT    # 16
LCH = LATENT // 128  # 8 contraction chunks

_CACHE = {}
DEBUG = False


def _build():
    import concourse.bacc as bacc
    from concourse import mybir
    from concourse.tile import TileContext

    f32 = mybir.dt.float32
    bf16 = mybir.dt.bfloat16
    f32r = mybir.dt.float32r
    Exp = mybir.ActivationFunctionType.Exp

    nc = bacc.Bacc("TRN2", target_bir_lowering=False, num_devices=NCORES)

    x_t = nc.declare_dram_parameter("x_t", [LATENT, S], f32r, isOutput=False)
    wq_t = nc.declare_dram_parameter("wq_t", [LATENT, DQ], f32r, isOutput=False)
    wk_d = nc.declare_dram_parameter("wk_d", [LATENT, 128], f32r, isOutput=False)
    wv_t = nc.declare_dram_parameter("wv_t", [LATENT, D], f32r, isOutput=False)
    wo_t = nc.declare_dram_parameter("wo_t", [LATENT, DQ], bf16, isOutput=False)
    mask = nc.declare_dram_parameter("mask", [128, 4 * 1024], f32r, isOutput=False)
    eye = nc.declare_dram_parameter("eye", [64, 64], f32, isOutput=False)
    out = nc.declare_dram_parameter("out", [DQ, S], f32, isOutput=True)
    dbg = {}
    if DEBUG:
        for nm, shp in (("dbg_pt", [128, 1024]), ("dbg_oa", [128, 512]),
                        ("dbg_rsm", [64, 512]), ("dbg_rcp", [64, 512]),
                        ("dbg_apc", [64, 512]), ("dbg_v", [128, 128]),
                        ("dbg_st", [128, 1024])):
            dbg[nm] = nc.declare_dram_parameter(nm, shp, f32, isOutput=True)

    RG = [[0, 1, 2, 3], [4, 5, 6, 7]]

    with TileContext(nc) as tc:
        with (
            tc.tile_pool(name="const", bufs=1) as cst,
            tc.tile_pool(name="sb", bufs=1) as sb,
            tc.tile_pool(name="ps", bufs=1, space="PSUM") as ps,
            tc.tile_pool(name="dram", bufs=1, space="DRAM") as dram,
        ):
            # ---- constants / weights ----
            ones_f = cst.tile([128, 64], f32)
            nc.vector.memset(ones_f[:], 1.0)
            ones_r = cst.tile([128, 64], f32r)
            nc.scalar.copy(ones_r[:], ones_f[:])
            # preload the exp table set early (overlaps weight DMAs)
            dummy = cst.tile([128, 8], f32)
            nc.scalar.activation(dummy[:], ones_f[:, :8], Exp)

            wq_sb = cst.tile([128, LCH, DQ], f32r)
            nc.sync.dma_start(wq_sb[:], wq_t[:].rearrange("(l p) m -> p l m", p=128))
            wk_sb = cst.tile([128, LCH, 128], f32r)
            nc.sync.dma_start(wk_sb[:], wk_d[:].rearrange("(l p) m -> p l m", p=128))
            wv_sb = cst.tile([128, LCH, D], f32r)
            nc.sync.dma_start(wv_sb[:], wv_t[:].rearrange("(l p) m -> p l m", p=128))
            eye_sb = cst.tile([64, 64], f32)
            nc.sync.dma_start(eye_sb[:], eye[:])
            mask_sb = cst.tile([128, 4 * 1024], f32r)
            nc.sync.dma_start(mask_sb[:], mask[:])
            wo_sb = cst.tile([128, LCH, DQ], bf16)
            nc.sync.dma_start(wo_sb[:], wo_t[:].rearrange("(l p) m -> p l m", p=128))

            # ---- persistent activations ----
            qT0 = sb.tile([128, S], f32r)   # heads 0,1 (rows 0:64 / 64:128)
            qT1 = sb.tile([128, S], f32r)   # heads 2,3
            qT_sb = [qT0, qT1]
            kT_sb = sb.tile([128, S], f32r)  # duplicated kT (rows 64:128 copy)
            # v tile t at [:, t, 0:64] (seq-major); [:, t, 64:128] = ones so the
            # fused OT matmul also produces the softmax rowsum in rows 64:128
            v_sb = sb.tile([128, NKT, 128], f32r)
            for t in range(NKT):
                nc.vector.tensor_copy(v_sb[:, t, 64:128], ones_f[:])

            # DRAM bounce buffers for the per-qblock AllGather
            agin = [dram.tile([DQ, QBLK], bf16, name=f"agin{j}") for j in range(NQB)]
            agout = [dram.tile([TP * DQ, QBLK], bf16, name=f"agout{j}")
                     for j in range(NQB)]

            # ---- phase 1: QKV projection (per q block) ----
            for j in range(NQB):
                qs = slice(QBLK * j, QBLK * (j + 1))
                xc = sb.tile([128, LCH, QBLK], f32r, tag="xc", bufs=2)
                xr = x_t[:].rearrange("(l p) s -> p l s", p=128)
                for l in range(LCH):
                    nc.sync.dma_start(xc[:, l, :], xr[:, l, qs])
                for c in range(2):
                    qps = ps.tile([128, QBLK], f32, tag="mm512", bufs=2)
                    for l in range(LCH):
                        nc.tensor.matmul(
                            qps[:],
                            wq_sb[:, l, 128 * c:128 * (c + 1)],
                            xc[:, l, :],
                            start=(l == 0),
                            stop=(l == LCH - 1),
                        )
                    nc.vector.tensor_copy(qT_sb[c][:, qs], qps[:])
                kps = ps.tile([128, QBLK], f32, tag="mm512", bufs=2)
                for l in range(LCH):
                    nc.tensor.matmul(
                        kps[:],
                        wk_sb[:, l, :],
                        xc[:, l, :],
                        start=(l == 0),
                        stop=(l == LCH - 1),
                    )
                nc.vector.tensor_copy(kT_sb[:, qs], kps[:])
                vtp = ps.tile([128, QBLK], f32, tag="mm512", bufs=2)
                for l in range(LCH):
                    nc.tensor.matmul(
                        vtp[0:D, :],
                        wv_sb[:, l, :],
                        xc[:, l, :],
                        start=(l == 0),
                        stop=(l == LCH - 1),
                    )
                vt_sb = sb.tile([64, QBLK], f32, tag="vt", bufs=2)
                nc.vector.tensor_copy(vt_sb[:], vtp[0:D, :])
                for si in range(QBLK // 128):
                    st_glob = (QBLK // 128) * j + si
                    vps = ps.tile([128, D], f32, tag="mm512", bufs=2)
                    nc.tensor.transpose(
                        vps[:], vt_sb[:, 128 * si:128 * (si + 1)], eye_sb[:]
                    )
                    nc.vector.tensor_copy(v_sb[:, st_glob, 0:D], vps[:])
                    if DEBUG and st_glob == 0:
                        vf = sb.tile([128, 128], f32, name="vf")
                        nc.vector.tensor_copy(vf[:], v_sb[:, 0, :])
                        nc.sync.dma_start(dbg["dbg_v"][:], vf[:])

            # ---- phase 2+3: attention, AllGather, output projection ----
            # Emission order pipelines proj(j-1) AFTER attention(j): the PE
            # instruction queue is FIFO, so proj matmuls must sit behind the
            # next attention block or every AllGather stalls the PE.
            def attention(j):
                qs = slice(QBLK * j, QBLK * (j + 1))
                nkt_j = (QBLK // KT) * (j + 1)
                for p in range(2):
                    # fused OT+rowsum accumulators, one bank per head:
                    # rows 0:64 = V.T @ P.T (attention out), 64:128 = rowsum
                    oa = ps.tile([128, QBLK], f32, tag="otrs", bufs=2, name="oa")
                    ob = ps.tile([128, QBLK], f32, tag="otrs", bufs=2, name="ob")
                    for t in range(nkt_j):
                        ks = slice(KT * t, KT * (t + 1))
                        st = ps.tile([128, 2 * QBLK], f32, tag="st", bufs=2,
                                     name="st")
                        nc.tensor.matmul(
                            st[:, 0:QBLK], kT_sb[0:64, ks], qT_sb[p][0:64, qs],
                            start=True, stop=True, tile_position=(0, 0),
                        )
                        nc.tensor.matmul(
                            st[:, QBLK:2 * QBLK], kT_sb[64:128, ks],
                            qT_sb[p][64:128, qs],
                            start=True, stop=True, tile_position=(64, 0),
                        )
                        if DEBUG and j == 0 and p == 0 and t == 0:
                            stf = sb.tile([128, 2 * QBLK], f32, name="stf")
                            nc.vector.tensor_copy(stf[:], st[:])
                            nc.sync.dma_start(dbg["dbg_st"][:], stf[:])
                        tl = t - (QBLK // KT) * j  # >=0 on the diagonal strip
                        if tl >= 0:
                            ptr = sb.tile([128, 2 * QBLK], f32r, tag="ptraw",
                                          bufs=2, name="ptr")
                            nc.scalar.activation(ptr[:], st[:], Exp, scale=SCALE)
                            pt = sb.tile([128, 2 * QBLK], f32r, tag="pt", bufs=4,
                                         name="pt")
                            nc.vector.tensor_mul(
                                pt[:], ptr[:],
                                mask_sb[:, 1024 * tl:1024 * (tl + 1)],
                            )
                        else:
                            pt = sb.tile([128, 2 * QBLK], f32r, tag="pt", bufs=4,
                                         name="pt")
                            nc.scalar.activation(pt[:], st[:], Exp, scale=SCALE)
                        if DEBUG and j == 0 and p == 0 and t == 0:
                            ptf = sb.tile([128, 2 * QBLK], f32, name="ptf")
                            nc.vector.tensor_copy(ptf[:], pt[:])
                            nc.sync.dma_start(dbg["dbg_pt"][:], ptf[:])
                        first, last = (t == 0), (t == nkt_j - 1)
                        nc.tensor.matmul(
                            oa[:], v_sb[:, t, :], pt[:, 0:QBLK],
                            start=first, stop=last,
                        )
                        nc.tensor.matmul(
                            ob[:], v_sb[:, t, :], pt[:, QBLK:2 * QBLK],
                            start=first, stop=last,
                        )
                    if DEBUG and j == 0 and p == 0:
                        oaf = sb.tile([128, QBLK], f32, name="oaf")
                        nc.vector.tensor_copy(oaf[:], oa[:])
                        nc.sync.dma_start(dbg["dbg_oa"][:], oaf[:])
                    for hb, oh in ((0, oa), (1, ob)):
                        h = 2 * p + hb
                        rsm = sb.tile([128, QBLK], f32, tag="rsm", bufs=2,
                                      name="rsm")
                        # psum -> sbuf first: custom DVE ops can't read PSUM
                        nc.vector.tensor_copy(rsm[64:128, :], oh[64:128, :])
                        # cross-partition move 64:128 -> 0:64 (DMA only path);
                        # approx-recip then runs at base partition 0 (it
                        # mis-executes on base-64 slices)
                        nc.sync.dma_start(rsm[0:64, :], rsm[64:128, :])
                        rcp = sb.tile([128, QBLK], f32, tag="rcp", bufs=2,
                                      name="rcp")
                        nc.vector.reciprocal_approx_fast(
                            out=rcp[0:64, :], in_=rsm[0:64, :])
                        if DEBUG and j == 0 and h == 0:
                            nc.sync.dma_start(dbg["dbg_rsm"][:], rsm[64:128, :])
                        apc = sb.tile([64, QBLK], bf16, tag="apc", bufs=2,
                                      name="apc")
                        nc.vector.tensor_mul(apc[:], oh[0:64, :], rcp[0:64, :])
                        if DEBUG and j == 0 and h == 0:
                            nc.sync.dma_start(dbg["dbg_rcp"][:], rcp[0:64, :])
                            apf = sb.tile([64, QBLK], f32, name="apf")
                            nc.vector.tensor_copy(apf[:], apc[:])
                            nc.sync.dma_start(dbg["dbg_apc"][:], apf[:])
                        nc.sync.dma_start(agin[j][64 * h:64 * (h + 1), :], apc[:])

                nc.gpsimd.collective_compute(
                    "AllGather",
                    mybir.AluOpType.bypass,
                    replica_groups=RG,
                    ins=[agin[j][:].opt()],
                    outs=[agout[j][:].opt()],
                )

            def out_proj(j):
                qs = slice(QBLK * j, QBLK * (j + 1))
                agsb = sb.tile([128, LCH, QBLK], bf16, tag="agsb", bufs=2,
                               name="agsb")
                agr = agout[j][:].rearrange("(l p) s -> p l s", p=128)
                for l in range(LCH):
                    nc.sync.dma_start(agsb[:, l, :], agr[:, l, :])
                for n in range(2):
                    pp = ps.tile([128, QBLK], f32, tag="mm512", bufs=2, name="pp")
                    for i in range(LCH):
                        nc.tensor.matmul(
                            pp[:],
                            wo_sb[:, i, 128 * n:128 * (n + 1)],
                            agsb[:, i, :],
                            start=(i == 0),
                            stop=(i == LCH - 1),
                        )
                    opc = sb.tile([128, QBLK], f32, tag="opc", bufs=2, name="opc")
                    nc.vector.tensor_copy(opc[:], pp[:])
                    nc.sync.dma_start(out[128 * n:128 * (n + 1), qs], opc[:])

            attention(0)
            attention(1)
            attention(2)
            out_proj(0)
            attention(3)
            out_proj(1)
            out_proj(2)
            out_proj(3)

    nc.finalize()
    return nc


def _shard_inputs(x, w_qkv, w_out):
    """Build the per-core input maps (host-side sharding only)."""
    x = np.asarray(x, dtype=np.float32)
    w_qkv = np.asarray(w_qkv, dtype=np.float32)
    w_out = np.asarray(w_out, dtype=np.float32)

    # causal masks for the 4 diagonal k-tile offsets, replicated for the
    # two heads packed side by side in each 1024-wide strip
    kk = np.arange(128)[:, None]
    qq = np.arange(QBLK)[None, :]
    strips = []
    for t in range(4):
        m = (kk <= qq - 128 * t).astype(np.float32)  # [128, 512]
        strips.append(np.concatenate([m, m], axis=1))  # [128, 1024]
    mask = np.ascontiguousarray(np.concatenate(strips, axis=1))  # [128, 4096]

    in_maps = []
    for c in range(NCORES):
        b, r = divmod(c, TP)
        wq = w_qkv[DQ * r:DQ * (r + 1), :]                    # [256, 1024]
        wk = w_qkv[H * D + D * r:H * D + D * (r + 1), :]      # [64, 1024]
        wv = w_qkv[(H + HK) * D + D * r:(H + HK) * D + D * (r + 1), :]
        wo = w_out[DQ * r:DQ * (r + 1), :]                    # [256, 1024]
        in_maps.append({
            "eye": np.eye(64, dtype=np.float32),
            "x_t": np.ascontiguousarray(x[b].T),
            "wq_t": np.ascontiguousarray(wq.T),
            "wk_d": np.ascontiguousarray(
                np.concatenate([wk.T, wk.T], axis=1)),
            "wv_t": np.ascontiguousarray(wv.T),
            "wo_t": np.ascontiguousarray(wo.T).astype(ml_dtypes.bfloat16),
            "mask": mask,
        })
    return in_maps


def _get_nc():
    if "nc" not in _CACHE:
        _CACHE["nc"] = _build()
    return _CACHE["nc"]


def _install_ntff_shim():
    """Make BASS_TRACE work under axon (antenv.axon_hooks is absent here)."""
    import types
    if "antenv.axon_hooks" in sys.modules:
        return True
    try:
        import antenv
        from trn_agent_boot.trn_boot import _ntff_profile_via_ctypes
        hook = _ntff_profile_via_ctypes("/opt/axon/libaxon_pjrt.so")
        if hook is None:
            return False
        mod = types.ModuleType("antenv.axon_hooks")
        state = {"hook": hook}
        mod.set_axon_ntff_profile_hook = lambda h: state.__setitem__("hook", h)
        mod.get_axon_ntff_profile_hook = lambda: state["hook"]
        sys.modules["antenv.axon_hooks"] = mod
        antenv.axon_hooks = mod
        return True
    except Exception:
        return False


LAST_RESULT = None


def kernel(x, w_qkv, w_out):
    global LAST_RESULT
    from concourse.bass_utils import run_bass_kernel_spmd

    nc = _get_nc()
    in_maps = _shard_inputs(x, w_qkv, w_out)

    trace = bool(os.environ.get("BASS_TRACE"))
    if trace:
        trace = _install_ntff_shim()
    kwargs = {}
    if trace and os.environ.get("BASS_TRACE_CORES") == "all":
        kwargs["trace_cores"] = list(range(NCORES))
    res = run_bass_kernel_spmd(
        nc, in_maps, core_ids=list(range(NCORES)), trace=trace, **kwargs
    )
    LAST_RESULT = res

    full = np.empty((B, S, LATENT), dtype=np.float32)
    for c in range(NCORES):
        b, r = divmod(c, TP)
        full[b, :, DQ * r:DQ * (r + 1)] = res.results[c]["out"].T
    return full
